# revision 1
# baseline (speedup 1.0000x reference)
"""AdaptiveQuantizationPatchGenerator — Trainium2 SPMD kernel.

Math identity used throughout: the reference gathers patch values at
windows (y0..y0+32, x0..x0+32) and scatter-adds them back at the SAME
windows, so the whole scatter reduces to

    out[b] = x[b] + count_b * patches[b]

where count_b[i,j] = #patches of sample b whose 32x32 window covers
pixel (i,j)  (separable: sum of 4 outer products of row/col indicator
vectors).  No data-dependent scatter is needed on device.

Distribution: pure data parallel, batch 32 -> 4 samples per core on 8
NeuronCores.  The elementwise combine runs on device via
run_bass_kernel_spmd; the conv stack / position MLP run host-side in
float32 numpy (BLAS).  If the device path is unavailable the combine
falls back to numpy so the output is always produced.
"""

import numpy as np

B, C, H, W = 32, 3, 256, 256
P = 32
NP = 4
STRENGTH = 0.1
N_CORES = 8
PER = B // N_CORES           # 4 samples per core
FLAT = PER * C * H * W       # 786432 = 128 * 6144
PARTS = 128
FREE = FLAT // PARTS         # 6144

LAST_EXEC_NS = None          # wall-clock of the device dispatch, for test.py


def _conv2d(x, w, b):
    """NCHW 3x3 stride-1 SAME correlation, float32, via im2col + sgemm."""
    Bn, Ci, Hh, Ww = x.shape
    xp = np.pad(x, ((0, 0), (0, 0), (1, 1), (1, 1)))
    s = xp.strides
    win = np.lib.stride_tricks.as_strided(
        xp, (Bn, Ci, 3, 3, Hh, Ww), (s[0], s[1], s[2], s[3], s[2], s[3]))
    y = np.einsum('ocuv,bcuvij->boij', w, win, optimize=True)
    return (y + b[None, :, None, None]).astype(np.float32)


def _sigmoid(v):
    return (1.0 / (1.0 + np.exp(-v.astype(np.float32)))).astype(np.float32)


def _host_patches_and_mask(x, w1, b1, w2, b2, w3, b3, pw1, pb1, pw2, pb2):
    h1 = np.maximum(_conv2d(x, w1, b1), 0.0).astype(np.float32)
    h2 = np.maximum(_conv2d(h1, w2, b2), 0.0).astype(np.float32)
    patches = (np.tanh(_conv2d(h2, w3, b3)) * STRENGTH).astype(np.float32)

    pooled = x.reshape(B, C, 8, H // 8, 8, W // 8).mean(axis=(3, 5),
                                                        dtype=np.float32)
    feat = pooled.reshape(B, -1).astype(np.float32)
    hmid = np.maximum(feat @ pw1.T + pb1, 0.0).astype(np.float32)
    pos = _sigmoid(hmid @ pw2.T + pb2).reshape(B, NP, 2)

    y0 = np.floor(pos[..., 0] * (H - P)).astype(np.int32)   # [B,NP]
    x0 = np.floor(pos[..., 1] * (W - P)).astype(np.int32)

    ar = np.arange(H, dtype=np.int32)
    rows = ((ar[None, None, :] >= y0[:, :, None])
            & (ar[None, None, :] < y0[:, :, None] + P)).astype(np.float32)
    cols = ((ar[None, None, :] >= x0[:, :, None])
            & (ar[None, None, :] < x0[:, :, None] + P)).astype(np.float32)
    count = np.einsum('bpi,bpj->bij', rows, cols).astype(np.float32)
    return patches, count


def _build_combine_graph():
    import concourse.bass as bass
    import concourse.mybir as mybir

    f32 = mybir.dt.float32
    nc = bass.Bass(target_bir_lowering=False, debug=False)
    x_ext = nc.declare_dram_parameter("x", [PARTS, FREE], f32, isOutput=False)
    a_ext = nc.declare_dram_parameter("addend", [PARTS, FREE], f32,
                                      isOutput=False)
    out_ext = nc.declare_dram_parameter("out", [PARTS, FREE], f32,
                                        isOutput=True)

    with (
        nc.sbuf_tensor("xt", [PARTS, FREE], f32) as xt,
        nc.sbuf_tensor("at", [PARTS, FREE], f32) as at,
        nc.sbuf_tensor("ot", [PARTS, FREE], f32) as ot,
        nc.semaphore("dma_sem") as dma_sem,
        nc.semaphore("v_sem") as v_sem,
        nc.Block() as block,
    ):
        # Whole-tensor transfers only: every wait_ge threshold is reached
        # exclusively by the exact set of DMAs it needs, so completion
        # order across DMA queues cannot race.
        @block.sync
        def _(sync):
            sync.dma_start(out=xt[:, :], in_=x_ext[:, :]).then_inc(dma_sem, 16)
            sync.dma_start(out=at[:, :], in_=a_ext[:, :]).then_inc(dma_sem, 16)
            sync.wait_ge(v_sem, 1)
            sync.dma_start(out=out_ext[:, :], in_=ot[:, :]).then_inc(
                dma_sem, 16)
            sync.wait_ge(dma_sem, 48)

        @block.vector
        def _(vector):
            vector.wait_ge(dma_sem, 32)
            vector.tensor_add(ot[:, :], xt[:, :], at[:, :]).then_inc(v_sem, 1)

    return nc


def _device_combine(x, addend):
    """out = x + addend on 8 NeuronCores, batch-sharded."""
    global LAST_EXEC_NS
    import time
    from concourse.bass_utils import run_bass_kernel_spmd

    nc = _build_combine_graph()
    in_maps = []
    for c in range(N_CORES):
        xs = np.ascontiguousarray(
            x[c * PER:(c + 1) * PER]).reshape(PARTS, FREE)
        as_ = np.ascontiguousarray(
            addend[c * PER:(c + 1) * PER]).reshape(PARTS, FREE)
        in_maps.append({"x": xs, "addend": as_})

    t0 = time.perf_counter_ns()
    res = run_bass_kernel_spmd(nc, in_maps, core_ids=list(range(N_CORES)))
    LAST_EXEC_NS = time.perf_counter_ns() - t0

    shards = [np.asarray(res.results[c]["out"]).reshape(PER, C, H, W)
              for c in range(N_CORES)]
    return np.concatenate(shards, axis=0)


def kernel(x, w1, b1, w2, b2, w3, b3, pw1, pb1, pw2, pb2, bit_width):
    x = np.asarray(x, dtype=np.float32)
    args = [np.asarray(a, dtype=np.float32)
            for a in (w1, b1, w2, b2, w3, b3, pw1, pb1, pw2, pb2)]
    patches, count = _host_patches_and_mask(x, *args)
    addend = (count[:, None, :, :] * patches).astype(np.float32)
    try:
        out = _device_combine(x, addend)
    except Exception:
        out = x + addend
    return out.astype(np.float32)



# revision 3
# speedup vs baseline: 1.6985x; 1.6985x over previous
"""AdaptiveQuantizationPatchGenerator — Trainium2 SPMD kernel (windowed).

Key identity: the reference gathers patch values at NP=4 32x32 windows
per sample and scatter-adds them back at the SAME windows, so conv
outputs are only ever USED inside those windows (+3px conv halo).  We
therefore run the 3-conv patch generator only on the gathered 38x38
windows on device — a ~16x FLOP cut and, far more importantly here, a
~350x cut in host<->device traffic (the axon tunnel moves ~30 MB/s, so
shipping the full 25 MB x / 25 MB out dominated the baseline).

Split:
  host   — position MLP (pooled features -> y0,x0), window gather,
           final out = x; out[windows] += 0.1 * pv  (scatter, trivial)
  device — the entire conv stack on all 128 windows, 8 cores, data
           parallel: core c gets samples 4c..4c+3 (16 windows).

Device layout (per core): 4 groups, one per sample; each group stacks
its 4 windows' channels on partitions with block-diagonal weights:
  conv1: contraction 12 (4w x 3ch),  out 128 (4w x 32ch)
  conv2: contraction 128 (4w x 32),  out  64 (4w x 16)
  conv3: contraction 64  (4w x 16),  out  12 (4w x 3)
Each conv is 9 tap-matmuls accumulated in PSUM over row-chunks, with
bias+ReLU (convs 1,2) / bias+Tanh (conv 3) fused into the PSUM->SBUF
activation copy.  The graph is input-independent, so the NEFF cache
hits on every call.
"""

import numpy as np

B, C, H, W = 32, 3, 256, 256
P = 32
NP = 4
STRENGTH = 0.1
N_CORES = 8
PER = B // N_CORES            # 4 samples per core
HALO = 3
WIN = P + 2 * HALO            # 38
C1, C2 = 32, 16

LAST_EXEC_NS = None           # wall-clock of the device dispatch, for test.py
LAST_HW_NS = None             # NTFF HW exec time when KTRACE=1


def _positions(x, pw1, pb1, pw2, pb2):
    """y0, x0 [B,NP] — must match the reference's float32 math."""
    pooled = x.reshape(B, C, 8, H // 8, 8, W // 8).mean(axis=(3, 5),
                                                        dtype=np.float32)
    feat = pooled.reshape(B, -1).astype(np.float32)
    hmid = np.maximum(feat @ pw1.T + pb1, 0.0).astype(np.float32)
    z = (hmid @ pw2.T + pb2).astype(np.float32)
    pos = (1.0 / (1.0 + np.exp(-z))).astype(np.float32).reshape(B, NP, 2)
    y0 = np.floor(pos[..., 0] * (H - P)).astype(np.int32)
    x0 = np.floor(pos[..., 1] * (W - P)).astype(np.int32)
    return y0, x0


_TAPS = [(dy, dx) for dy in range(3) for dx in range(3)]


def _build_graph():
    import concourse.bass as bass
    import concourse.mybir as mybir
    from concourse.tile import TileContext

    f32 = mybir.dt.float32
    Act = mybir.ActivationFunctionType

    nc = bass.Bass(target_bir_lowering=False, debug=False)
    xw = nc.declare_dram_parameter("xw", [PER * NP * C, WIN, WIN], f32,
                                   isOutput=False)
    w1c = nc.declare_dram_parameter("w1c", [C, 9, C1], f32, isOutput=False)
    w2c = nc.declare_dram_parameter("w2c", [C1, 9, C2], f32, isOutput=False)
    w3c = nc.declare_dram_parameter("w3c", [C2, 9, C], f32, isOutput=False)
    b1g = nc.declare_dram_parameter("b1g", [NP * C1, 1], f32, isOutput=False)
    b2g = nc.declare_dram_parameter("b2g", [NP * C2, 1], f32, isOutput=False)
    b3g = nc.declare_dram_parameter("b3g", [NP * C, 1], f32, isOutput=False)
    out = nc.declare_dram_parameter("out", [PER * NP * C, P, P], f32,
                                    isOutput=True)

    with TileContext(nc) as tc:
        with (
            tc.tile_pool(name="wpool", bufs=1) as wpool,
            tc.tile_pool(name="sb", bufs=2) as sb,
            tc.tile_pool(name="ps", bufs=2, space="PSUM") as ps,
        ):
            # Block-diagonal weights: zero once, then drop each window's
            # block on the diagonal straight from DRAM.
            w1sb = wpool.tile([NP * C, 9, NP * C1], f32)
            w2sb = wpool.tile([NP * C1, 9, NP * C2], f32)
            w3sb = wpool.tile([NP * C2, 9, NP * C], f32)
            nc.any.memzero(w1sb)
            nc.any.memzero(w2sb)
            nc.any.memzero(w3sb)
            for k in range(NP):
                nc.sync.dma_start(
                    out=w1sb[k * C:(k + 1) * C, :, k * C1:(k + 1) * C1],
                    in_=w1c)
                nc.sync.dma_start(
                    out=w2sb[k * C1:(k + 1) * C1, :, k * C2:(k + 1) * C2],
                    in_=w2c)
                nc.sync.dma_start(
                    out=w3sb[k * C2:(k + 1) * C2, :, k * C:(k + 1) * C],
                    in_=w3c)
            b1t = wpool.tile([NP * C1, 1], f32)
            b2t = wpool.tile([NP * C2, 1], f32)
            b3t = wpool.tile([NP * C, 1], f32)
            nc.sync.dma_start(out=b1t, in_=b1g)
            nc.sync.dma_start(out=b2t, in_=b2g)
            nc.sync.dma_start(out=b3t, in_=b3g)

            for g in range(PER):
                r = g * NP * C
                x_t = sb.tile([NP * C, WIN, WIN], f32, tag="x")
                nc.sync.dma_start(out=x_t, in_=xw[r:r + NP * C])

                # conv1: 38x38 -> 36x36
                h1 = sb.tile([NP * C1, 36, 36], f32, tag="h1")
                for r0, cr in [(0, 12), (12, 12), (24, 12)]:
                    pt = ps.tile([NP * C1, cr, 36], f32, tag="p1")
                    for t, (dy, dx) in enumerate(_TAPS):
                        nc.tensor.matmul(
                            pt, w1sb[:, t, :],
                            x_t[:, r0 + dy:r0 + dy + cr, dx:dx + 36],
                            start=(t == 0), stop=(t == 8))
                    nc.scalar.activation(h1[:, r0:r0 + cr, :], pt,
                                         Act.Relu, bias=b1t[:, 0:1])

                # conv2: 36x36 -> 34x34
                h2 = sb.tile([NP * C2, 34, 34], f32, tag="h2")
                for r0, cr in [(0, 12), (12, 12), (24, 10)]:
                    pt = ps.tile([NP * C2, cr, 34], f32, tag="p2")
                    for t, (dy, dx) in enumerate(_TAPS):
                        nc.tensor.matmul(
                            pt, w2sb[:, t, :],
                            h1[:, r0 + dy:r0 + dy + cr, dx:dx + 34],
                            start=(t == 0), stop=(t == 8))
                    nc.scalar.activation(h2[:, r0:r0 + cr, :], pt,
                                         Act.Relu, bias=b2t[:, 0:1])

                # conv3: 34x34 -> 32x32, tanh (x0.1 applied on host)
                pv = sb.tile([NP * C, P, P], f32, tag="pv")
                for r0, cr in [(0, 16), (16, 16)]:
                    pt = ps.tile([NP * C, cr, P], f32, tag="p3")
                    for t, (dy, dx) in enumerate(_TAPS):
                        nc.tensor.matmul(
                            pt, w3sb[:, t, :],
                            h2[:, r0 + dy:r0 + dy + cr, dx:dx + P],
                            start=(t == 0), stop=(t == 8))
                    nc.scalar.activation(pv[:, r0:r0 + cr, :], pt,
                                         Act.Tanh, bias=b3t[:, 0:1])

                nc.sync.dma_start(out=out[r:r + NP * C], in_=pv)
    return nc


def _pack_weights(w1, b1, w2, b2, w3, b3):
    # w[Co,Ci,3,3] -> [Ci, 9, Co] with tap index t = dy*3+dx
    w1c = np.ascontiguousarray(w1.transpose(1, 2, 3, 0).reshape(C, 9, C1))
    w2c = np.ascontiguousarray(w2.transpose(1, 2, 3, 0).reshape(C1, 9, C2))
    w3c = np.ascontiguousarray(w3.transpose(1, 2, 3, 0).reshape(C2, 9, C))
    b1g = np.ascontiguousarray(np.tile(b1, NP)[:, None])
    b2g = np.ascontiguousarray(np.tile(b2, NP)[:, None])
    b3g = np.ascontiguousarray(np.tile(b3, NP)[:, None])
    return w1c, w2c, w3c, b1g, b2g, b3g


def _device_patches(xwin, packed):
    """xwin [B,NP,C,WIN,WIN] -> pv [B,NP,C,P,P] = tanh(conv stack)."""
    global LAST_EXEC_NS, LAST_HW_NS
    import os
    import time
    from concourse.bass_utils import run_bass_kernel_spmd

    nc = _build_graph()
    w1c, w2c, w3c, b1g, b2g, b3g = packed
    in_maps = []
    for c in range(N_CORES):
        in_maps.append({
            "xw": np.ascontiguousarray(
                xwin[c * PER:(c + 1) * PER].reshape(PER * NP * C, WIN, WIN)),
            "w1c": w1c, "w2c": w2c, "w3c": w3c,
            "b1g": b1g, "b2g": b2g, "b3g": b3g,
        })

    trace = os.environ.get("KTRACE", "") == "1"
    t0 = time.perf_counter_ns()
    res = run_bass_kernel_spmd(nc, in_maps, core_ids=list(range(N_CORES)),
                               trace=trace)
    LAST_EXEC_NS = time.perf_counter_ns() - t0
    LAST_HW_NS = res.exec_time_ns

    pv = np.stack([np.asarray(res.results[c]["out"]).reshape(PER, NP, C, P, P)
                   for c in range(N_CORES)])
    return pv.reshape(B, NP, C, P, P)


def _host_patches(xwin, w1, b1, w2, b2, w3, b3):
    """Numpy fallback: valid convs on the gathered windows."""
    def vconv(xin, wgt, bias):
        n, ci, h, w = xin.shape
        o = np.zeros((n, wgt.shape[0], h - 2, w - 2), np.float32)
        for dy in range(3):
            for dx in range(3):
                o += np.einsum('oc,nchw->nohw', wgt[:, :, dy, dx],
                               xin[:, :, dy:dy + h - 2, dx:dx + w - 2],
                               optimize=True)
        return o + bias[None, :, None, None]

    xin = xwin.reshape(B * NP, C, WIN, WIN)
    h1 = np.maximum(vconv(xin, w1, b1), 0)
    h2 = np.maximum(vconv(h1, w2, b2), 0)
    return np.tanh(vconv(h2, w3, b3)).reshape(B, NP, C, P, P)


def kernel(x, w1, b1, w2, b2, w3, b3, pw1, pb1, pw2, pb2, bit_width):
    x = np.asarray(x, dtype=np.float32)
    w1, b1, w2, b2, w3, b3, pw1, pb1, pw2, pb2 = [
        np.asarray(a, dtype=np.float32)
        for a in (w1, b1, w2, b2, w3, b3, pw1, pb1, pw2, pb2)]

    y0, x0 = _positions(x, pw1, pb1, pw2, pb2)

    xpad = np.pad(x, ((0, 0), (0, 0), (HALO, HALO), (HALO, HALO)))
    xwin = np.empty((B, NP, C, WIN, WIN), np.float32)
    for b in range(B):
        for w in range(NP):
            xwin[b, w] = xpad[b, :, y0[b, w]:y0[b, w] + WIN,
                              x0[b, w]:x0[b, w] + WIN]

    try:
        pv = _device_patches(xwin, _pack_weights(w1, b1, w2, b2, w3, b3))
    except Exception:
        import traceback
        traceback.print_exc()
        pv = _host_patches(xwin, w1, b1, w2, b2, w3, b3)

    out = x.copy()
    for b in range(B):
        for w in range(NP):
            out[b, :, y0[b, w]:y0[b, w] + P,
                x0[b, w]:x0[b, w] + P] += STRENGTH * pv[b, w]
    return out
